# revision 12
# baseline (speedup 1.0000x reference)
"""Trainium2 Bass kernel for nn_CrossFusion (CBN + L2-norms + tiny-head cross-attention).

Self-contained: hardcodes shapes/sharding. Shards the S1 (query) axis across 8
NeuronCores; x2-side work (stats, k, v) is replicated per core. The attention
matrix is never materialized to HBM: scores are generated on the fly as
e = exp(q_s * k_t) with one ACT op per (s-chunk, head), the softmax denominator
comes free via the ACT accumulator, and the numerator is a fused
tensor_tensor_reduce against a broadcast v in bf16.

Layouts: all big tensors are column-form "(p c) d": row index = p*C + c, so a
[128, C*64] SBUF tile holds rows with partition p owning rows p*C..p*C+C-1.
The t-order of k/v/x2 and s-order of q/output use the same mapping, so the
softmax (a sum over all t) is order-invariant and outputs land contiguously.
"""
import numpy as np

S = 4096          # S1 == S2
D = 64
H = 2
NCORES = 8
SSH = S // NCORES  # 512 rows of s per core
SC = SSH // 128    # 4 s-chunks per core
TC = S // 128      # 32 t-chunks
EPS_BN = 1e-5

_CACHE = {}


def _build(split=True):
    import concourse.bass as bass
    import concourse.tile as tile
    import concourse.mybir as mybir

    f32 = mybir.dt.float32
    bf16 = mybir.dt.bfloat16
    AF = mybir.ActivationFunctionType
    ALU = mybir.AluOpType
    P = 128

    nc = bass.Bass("TRN2", target_bir_lowering=False, debug=False)

    x1s = nc.dram_tensor("x1s", [SSH, D], f32, kind="ExternalInput")
    x1f = nc.dram_tensor("x1f", [S, D], f32, kind="ExternalInput")
    x2 = nc.dram_tensor("x2", [S, D], f32, kind="ExternalInput")
    Wq = nc.dram_tensor("Wq", [D, H], f32, kind="ExternalInput")
    Wk = nc.dram_tensor("Wk", [D, H], f32, kind="ExternalInput")
    Wv = nc.dram_tensor("Wv", [D, H], f32, kind="ExternalInput")
    Wo = nc.dram_tensor("Wo", [H, 2], f32, kind="ExternalInput")
    bo = nc.dram_tensor("bo", [1, 2], f32, kind="ExternalInput")
    Wg1 = nc.dram_tensor("Wg1", [D, D], f32, kind="ExternalInput")
    Wg2 = nc.dram_tensor("Wg2", [D, D], f32, kind="ExternalInput")
    Wb1 = nc.dram_tensor("Wb1", [D, D], f32, kind="ExternalInput")
    Wb2 = nc.dram_tensor("Wb2", [D, D], f32, kind="ExternalInput")
    y = nc.dram_tensor("y", [SSH, 2], f32, kind="ExternalOutput")

    # DRAM scratch for partition-broadcast bounces
    k_dram = nc.dram_tensor("k_dram", [H, S], bf16)
    v_dram = nc.dram_tensor("v_dram", [H, S], bf16)

    with tile.TileContext(nc) as tc:
        with tc.tile_pool(name="big", bufs=1) as big, \
             tc.tile_pool(name="scr", bufs=4) as scrp, \
             tc.tile_pool(name="kv", bufs=1) as kvp, \
             tc.tile_pool(name="small", bufs=1) as sm, \
             tc.tile_pool(name="att", bufs=3) as att, \
             tc.tile_pool(name="psum", bufs=1, space="PSUM") as psum:

            # ---------- loads ----------
            x2big = big.tile([P, TC * D], f32)
            nc.sync.dma_start(x2big[:], x2.rearrange("(p c) d -> p (c d)", p=P))
            x1fbig = big.tile([P, TC * D], f32)
            nc.sync.dma_start(x1fbig[:], x1f.rearrange("(p c) d -> p (c d)", p=P))
            x1sbig = big.tile([P, SC * D], f32)
            nc.sync.dma_start(x1sbig[:], x1s.rearrange("(p c) d -> p (c d)", p=P))

            wg1 = sm.tile([D, D], f32)
            nc.gpsimd.dma_start(wg1[:], Wg1[:, :])
            wg2 = sm.tile([D, D], f32)
            nc.gpsimd.dma_start(wg2[:], Wg2[:, :])
            wb1 = sm.tile([D, D], f32)
            nc.gpsimd.dma_start(wb1[:], Wb1[:, :])
            wb2 = sm.tile([D, D], f32)
            nc.gpsimd.dma_start(wb2[:], Wb2[:, :])

            # Bundle every tiny partition-broadcast into ONE DMA: bounce the
            # qkv weight columns (transposed to contiguous rows), Wo (flat 4)
            # and bo (2) into one [1, 390] DRAM row, then one [128, 390]
            # broadcast. Saves ~2.4us of descriptor overhead per separate DMA.
            w_rows = nc.dram_tensor("w_rows", [1, 6 * D + 6], f32)
            for i, Wt in enumerate((Wq, Wk, Wv)):
                t = sm.tile([D, H], f32, name=f"wtmp{i}")
                nc.sync.dma_start(t[:], Wt[:, :])
                nc.sync.dma_start(
                    w_rows[0:1, i * 2 * D:(i + 1) * 2 * D].rearrange("1 (h d) -> d (1 h)", h=H), t[:])
            wotmp = sm.tile([H, 2], f32)
            nc.sync.dma_start(wotmp[:], Wo[:, :])
            nc.sync.dma_start(w_rows[0:1, 6 * D:6 * D + 4].rearrange("1 (h j) -> h (1 j)", h=H), wotmp[:])
            botmp = sm.tile([1, 2], f32)
            nc.sync.dma_start(botmp[:], bo[:, :])
            nc.sync.dma_start(w_rows[0:1, 6 * D + 4:6 * D + 6], botmp[:])

            wab = sm.tile([P, 6 * D + 6], f32)
            nc.sync.dma_start(wab[:], w_rows[0:1, :].to_broadcast((P, 6 * D + 6)))
            wq_b = [wab[:, h * D:(h + 1) * D] for h in range(H)]
            wk_b = [wab[:, (2 + h) * D:(3 + h) * D] for h in range(H)]
            wv_b = [wab[:, (4 + h) * D:(5 + h) * D] for h in range(H)]
            wo_b = {(h, j): wab[:, 6 * D + h * 2 + j:6 * D + h * 2 + j + 1]
                    for h in range(H) for j in range(2)}
            bo_b = [wab[:, 6 * D + 4 + j:6 * D + 4 + j + 1] for j in range(2)]

            ones = sm.tile([P, 1], f32)
            nc.vector.memset(ones[:], 1.0)

            # ---------- x1 mean -> h_col [64,1] ----------
            h_ps = psum.tile([D, 1], f32)
            for c in range(TC):
                nc.tensor.matmul(h_ps[:], x1fbig[:, c * D:(c + 1) * D], ones[:],
                                 start=(c == 0), stop=(c == TC - 1))
            h_col = sm.tile([D, 1], f32)
            nc.vector.tensor_scalar_mul(h_col[:], h_ps[:], 1.0 / S)

            # ---------- x2 stats: mu, E[x^2] ----------
            x2sq = big.tile([P, TC * D], f32)
            nc.gpsimd.tensor_tensor(out=x2sq[:], in0=x2big[:], in1=x2big[:], op=ALU.mult)

            mu_ps = psum.tile([1, D], f32)
            for c in range(TC):
                nc.tensor.matmul(mu_ps[:], ones[:], x2big[:, c * D:(c + 1) * D],
                                 start=(c == 0), stop=(c == TC - 1))
            msq_ps = psum.tile([1, D], f32)
            for c in range(TC):
                nc.tensor.matmul(msq_ps[:], ones[:], x2sq[:, c * D:(c + 1) * D],
                                 start=(c == 0), stop=(c == TC - 1))
            mu = sm.tile([1, D], f32)
            nc.vector.tensor_scalar_mul(mu[:], mu_ps[:], 1.0 / S)
            msq = sm.tile([1, D], f32)
            nc.vector.tensor_scalar_mul(msq[:], msq_ps[:], 1.0 / S)

            # var = msq - mu^2 ; rstd = sqrt(1/(var+eps))
            musq = sm.tile([1, D], f32)
            nc.vector.tensor_tensor(out=musq[:], in0=mu[:], in1=mu[:], op=ALU.mult)
            var = sm.tile([1, D], f32)
            nc.vector.tensor_tensor(out=var[:], in0=msq[:], in1=musq[:], op=ALU.subtract)
            nc.vector.tensor_scalar_add(var[:], var[:], EPS_BN)
            rvar = sm.tile([1, D], f32)
            nc.vector.reciprocal(rvar[:], var[:])
            rstd = sm.tile([1, D], f32)
            nc.scalar.activation(rstd[:], rvar[:], AF.Sqrt)

            # ---------- CBN MLPs: dg, db rows [1, 64] ----------
            def mlp(w1, w2, name):
                z_ps = psum.tile([D, 1], f32, name=f"z_ps_{name}")
                nc.tensor.matmul(z_ps[:], w1[:], h_col[:], start=True, stop=True)
                zr = sm.tile([D, 1], f32, name=f"zr_{name}")
                nc.scalar.activation(zr[:], z_ps[:], AF.Relu)
                d_ps = psum.tile([1, D], f32, name=f"d_ps_{name}")
                nc.tensor.matmul(d_ps[:], zr[:], w2[:], start=True, stop=True)
                return d_ps

            dg_ps = mlp(wg1, wg2, "g")
            db_ps = mlp(wb1, wb2, "b")

            # A = (1+dg)*rstd ; B = db - mu*A
            dgp1 = sm.tile([1, D], f32)
            nc.vector.tensor_scalar_add(dgp1[:], dg_ps[:], 1.0)
            A_row = sm.tile([1, D], f32)
            nc.vector.tensor_tensor(out=A_row[:], in0=dgp1[:], in1=rstd[:], op=ALU.mult)
            muA = sm.tile([1, D], f32)
            nc.vector.tensor_tensor(out=muA[:], in0=mu[:], in1=A_row[:], op=ALU.mult)
            B_row = sm.tile([1, D], f32)
            nc.vector.tensor_tensor(out=B_row[:], in0=db_ps[:], in1=muA[:], op=ALU.subtract)

            # bounce A,B through DRAM to partition-broadcast
            # broadcast A,B across partitions on PE: out = ones[1,128].T @ ab_row[1,128]
            ab_row = sm.tile([1, 2 * D], f32)
            nc.vector.tensor_copy(ab_row[:, 0:D], A_row[:])
            nc.vector.tensor_copy(ab_row[:, D:2 * D], B_row[:])
            ones_row = sm.tile([1, P], f32)
            nc.vector.memset(ones_row[:], 1.0)
            ab_ps = psum.tile([P, 2 * D], f32)
            nc.tensor.matmul(ab_ps[:], ones_row[:], ab_row[:], start=True, stop=True)
            ab_b = sm.tile([P, 2 * D], f32)
            nc.vector.tensor_copy(ab_b[:], ab_ps[:])
            A_b = ab_b[:, 0:D]
            B_b = ab_b[:, D:2 * D]

            # ---------- v2 = x2*A + B (CBN applied) ----------
            v2 = big.tile([P, TC * D], f32)
            x2v = x2big[:].rearrange("p (c d) -> p c d", d=D)
            v2v = v2[:].rearrange("p (c d) -> p c d", d=D)
            A_rep = A_b.rearrange("p (c d) -> p c d", c=1).to_broadcast((P, TC, D))
            B_rep = B_b.rearrange("p (c d) -> p c d", c=1).to_broadcast((P, TC, D))
            nc.gpsimd.tensor_tensor(out=v2v, in0=x2v, in1=A_rep, op=ALU.mult)
            nc.gpsimd.tensor_tensor(out=v2v, in0=v2v, in1=B_rep, op=ALU.add)

            # ---------- row norms ----------
            rn2 = sm.tile([P, TC], f32)
            nc.vector.reduce_sum(rn2[:], x2sq[:].rearrange("p (c d) -> p c d", d=D),
                                 axis=mybir.AxisListType.X)
            in2 = sm.tile([P, TC], f32)
            nc.vector.reciprocal(in2[:], rn2[:])
            nc.scalar.activation(in2[:], in2[:], AF.Sqrt)

            v2sq = big.tile([P, TC * D], f32)
            nc.gpsimd.tensor_tensor(out=v2sq[:], in0=v2[:], in1=v2[:], op=ALU.mult)
            rnv = sm.tile([P, TC], f32)
            nc.vector.reduce_sum(rnv[:], v2sq[:].rearrange("p (c d) -> p c d", d=D),
                                 axis=mybir.AxisListType.X)
            inv2 = sm.tile([P, TC], f32)
            nc.vector.reciprocal(inv2[:], rnv[:])
            nc.scalar.activation(inv2[:], inv2[:], AF.Sqrt)

            x1ssq = sm.tile([P, SC * D], f32)
            nc.vector.tensor_tensor(out=x1ssq[:], in0=x1sbig[:], in1=x1sbig[:], op=ALU.mult)
            rn1 = sm.tile([P, SC], f32)
            nc.vector.reduce_sum(rn1[:], x1ssq[:].rearrange("p (c d) -> p c d", d=D),
                                 axis=mybir.AxisListType.X)
            in1 = sm.tile([P, SC], f32)
            nc.vector.reciprocal(in1[:], rn1[:])
            nc.scalar.activation(in1[:], in1[:], AF.Sqrt)

            # ---------- projections (col-form mul+reduce) ----------
            def proj(src_big, w_b, inv, n_chunks, name, out_dt=f32, meng=None):
                meng = meng or nc.vector
                """out[p, c] = inv[p,c] * sum_d src[p, c, d] * w_b[p, d]"""
                scr = scrp.tile([P, n_chunks * D], f32, tag="scr", name=f"scr_{name}")
                w_rep = w_b.rearrange("p (c d) -> p c d", c=1).to_broadcast((P, n_chunks, D))
                meng.tensor_tensor(out=scr[:].rearrange("p (c d) -> p c d", d=D),
                                   in0=src_big[:].rearrange("p (c d) -> p c d", d=D),
                                   in1=w_rep, op=ALU.mult)
                raw = sm.tile([P, n_chunks], f32, name=f"raw_{name}")
                nc.vector.reduce_sum(raw[:], scr[:].rearrange("p (c d) -> p c d", d=D),
                                     axis=mybir.AxisListType.X)
                outp = sm.tile([P, n_chunks], out_dt, name=f"proj_{name}")
                nc.vector.tensor_tensor(out=outp[:], in0=raw[:], in1=inv[:], op=ALU.mult)
                return outp

            q_hat = [proj(x1sbig, wq_b[h], in1, SC, f"q{h}") for h in range(H)]
            k_hat = [proj(x2big, wk_b[h], in2, TC, f"k{h}", out_dt=bf16, meng=nc.gpsimd) for h in range(H)]
            v_hat = [proj(v2, wv_b[h], inv2, TC, f"v{h}", out_dt=bf16) for h in range(H)]

            # ---------- broadcast k and v (bf16) across partitions ----------
            # bf16 halves the broadcast bytes; spread across SP/PE HW queues.
            k_b = []
            v_b = []
            for h in range(H):
                nc.sync.dma_start(k_dram[h:h + 1, :], k_hat[h][:])
                nc.sync.dma_start(v_dram[h:h + 1, :], v_hat[h][:])
                kb = kvp.tile([P, S], bf16, name=f"k_b{h}")
                eng = nc.sync if h == 0 else nc.gpsimd
                eng.dma_start(kb[:], k_dram[h:h + 1, :].to_broadcast((P, S)))
                k_b.append(kb)
                vb = kvp.tile([P, S], bf16, name=f"v_b{h}")
                eng = nc.sync if h == 0 else nc.gpsimd
                eng.dma_start(vb[:], v_dram[h:h + 1, :].to_broadcast((P, S)))
                v_b.append(vb)

            # ---------- attention: per (head, s-chunk) ----------
            r_cols = {}
            for h in range(H):
                for sc in range(SC):
                    e_t = att.tile([P, S], bf16, tag="e", name=f"e_{h}_{sc}")
                    den = sm.tile([P, 1], f32, name=f"den_{h}_{sc}")
                    nc.scalar.activation(e_t[:], k_b[h][:], AF.Exp,
                                         bias=0.0, scale=q_hat[h][:, sc:sc + 1],
                                         accum_out=den[:])
                    scr = att.tile([P, S], bf16, tag="ttr_scr", name=f"ts_{h}_{sc}")
                    num = sm.tile([P, 1], f32, name=f"num_{h}_{sc}")
                    nc.vector.tensor_tensor(out=scr[:], in0=e_t[:], in1=v_b[h][:], op=ALU.mult)
                    h1 = att.tile([P, S // 2], bf16, tag="h1", name=f"h1_{h}_{sc}")
                    nc.vector.tensor_tensor(out=h1[:], in0=scr[:, :S // 2], in1=scr[:, S // 2:], op=ALU.add)
                    h2 = att.tile([P, S // 4], bf16, tag="h2", name=f"h2_{h}_{sc}")
                    nc.vector.tensor_tensor(out=h2[:], in0=h1[:, :S // 4], in1=h1[:, S // 4:], op=ALU.add)
                    nc.vector.reduce_sum(num[:], h2[:], axis=mybir.AxisListType.X)
                    rden = sm.tile([P, 1], f32, name=f"rd_{h}_{sc}")
                    nc.vector.reciprocal(rden[:], den[:])
                    r = sm.tile([P, 1], f32, name=f"r_{h}_{sc}")
                    nc.vector.tensor_tensor(out=r[:], in0=num[:], in1=rden[:], op=ALU.mult)
                    r_cols[(h, sc)] = r

            # ---------- epilogue: logits -> sigmoid -> out ----------
            z_all = sm.tile([P, SC * 2], f32)
            for sc in range(SC):
                for j in range(2):
                    zc = z_all[:, sc * 2 + j:sc * 2 + j + 1]
                    nc.vector.tensor_scalar(out=zc, in0=r_cols[(0, sc)][:],
                                            scalar1=wo_b[(0, j)], scalar2=bo_b[j],
                                            op0=ALU.mult, op1=ALU.add)
                    t2 = sm.tile([P, 1], f32, name=f"t2_{sc}_{j}")
                    nc.vector.tensor_scalar_mul(t2[:], r_cols[(1, sc)][:], wo_b[(1, j)])
                    nc.vector.tensor_tensor(out=zc, in0=zc, in1=t2[:], op=ALU.add)

            # sigmoid(z) = 1/(1+exp(-z))  (reuses the exp table set)
            sig = sm.tile([P, SC * 2], f32)
            nc.scalar.activation(sig[:], z_all[:], AF.Exp, bias=0.0, scale=-1.0)
            nc.vector.tensor_scalar_add(sig[:], sig[:], 1.0)
            nc.vector.reciprocal(sig[:], sig[:])

            nc.sync.dma_start(y.rearrange("(p c) j -> p (c j)", p=P), sig[:])

    if split:
        _split_waits(nc, mybir)
    return nc


def _split_waits(nc, mybir, maxw=1):
    """This container's walrus build rejects instructions carrying more than
    ~2 sync-wait commands. Split excess waits onto zero-register-write nops
    inserted just before the instruction on the same engine (same-engine
    program order preserves the wait-before-execute semantics)."""
    ctr = 0
    for bb in nc.m.functions[0].blocks:
        new = []
        for inst in bb.instructions:
            si = inst.sync_info
            if si is not None and si.on_wait and len(si.on_wait) > maxw:
                waits = list(si.on_wait)
                ename = str(inst.engine).split(".")[-1]
                for w in waits[:-maxw]:
                    ctr += 1
                    new.append(mybir.InstRegisterMove(
                        name=f"WS-{ctr}",
                        ins=[mybir.ImmediateValue(kind="imm_value", dtype=mybir.dt.int32, value=0)],
                        outs=[mybir.RegisterAccess(kind="register_access", regref=f"{ename}_zero", dtype=mybir.dt.int32)],
                        engine=inst.engine,
                        sync_info=mybir.SyncInfo(on_wait=[w], on_update=[]),
                    ))
                si.on_wait = waits[-maxw:]
            new.append(inst)
        bb.instructions = new


def _get_program():
    if "nc" not in _CACHE:
        _CACHE["nc"] = _build()
    return _CACHE["nc"]


def kernel(x1, x2, Wq, Wk, Wv, Wo, bo, Wg1, Wg2, Wb1, Wb2):
    from concourse import bass_utils

    nc = _get_program()
    x1s_full = np.ascontiguousarray(x1[0])  # [4096, 64]
    x2s = np.ascontiguousarray(x2[0])

    in_maps = []
    for i in range(NCORES):
        in_maps.append({
            "x1s": np.ascontiguousarray(x1s_full[i * SSH:(i + 1) * SSH]),
            "x1f": x1s_full,
            "x2": x2s,
            "Wq": Wq, "Wk": Wk, "Wv": Wv, "Wo": Wo,
            "bo": np.ascontiguousarray(bo[None, :]),
            "Wg1": Wg1, "Wg2": Wg2, "Wb1": Wb1, "Wb2": Wb2,
        })

    # First execution of a freshly-compiled NEFF occasionally reports a
    # transient device error through the PJRT proxy; a retry succeeds.
    last_err = None
    for attempt in range(3):
        try:
            res = bass_utils.run_bass_kernel_spmd(nc, in_maps, core_ids=list(range(NCORES)))
            out = np.concatenate([res.results[i]["y"] for i in range(NCORES)], axis=0)
            return out.reshape(1, S, 2)
        except Exception as e:  # noqa: BLE001
            last_err = e
            import time
            time.sleep(5)
    raise last_err


# revision 19
# speedup vs baseline: 1.0235x; 1.0235x over previous
"""Trainium2 Bass kernel for nn_CrossFusion (CBN + L2-norms + tiny-head cross-attention).

Self-contained: hardcodes shapes/sharding. Shards the S1 (query) axis across 8
NeuronCores; x2-side work (stats, k, v) is replicated per core. The attention
matrix is never materialized to HBM: scores are generated on the fly as
e = exp(q_s * k_t) with one ACT op per (s-chunk, head), the softmax denominator
comes free via the ACT accumulator, and the numerator is a fused
tensor_tensor_reduce against a broadcast v in bf16.

Layouts: all big tensors are column-form "(p c) d": row index = p*C + c, so a
[128, C*64] SBUF tile holds rows with partition p owning rows p*C..p*C+C-1.
The t-order of k/v/x2 and s-order of q/output use the same mapping, so the
softmax (a sum over all t) is order-invariant and outputs land contiguously.
"""
import numpy as np

S = 4096          # S1 == S2
D = 64
H = 2
NCORES = 8
SSH = S // NCORES  # 512 rows of s per core
SC = SSH // 128    # 4 s-chunks per core
TC = S // 128      # 32 t-chunks
EPS_BN = 1e-5

_CACHE = {}


def _build(split=True):
    import concourse.bass as bass
    import concourse.tile as tile
    import concourse.mybir as mybir

    f32 = mybir.dt.float32
    bf16 = mybir.dt.bfloat16
    AF = mybir.ActivationFunctionType
    ALU = mybir.AluOpType
    P = 128

    nc = bass.Bass("TRN2", target_bir_lowering=False, debug=False)

    x1s = nc.dram_tensor("x1s", [SSH, D], f32, kind="ExternalInput")
    x1f = nc.dram_tensor("x1f", [S, D], f32, kind="ExternalInput")
    x2 = nc.dram_tensor("x2", [S, D], f32, kind="ExternalInput")
    Wq = nc.dram_tensor("Wq", [D, H], f32, kind="ExternalInput")
    Wk = nc.dram_tensor("Wk", [D, H], f32, kind="ExternalInput")
    Wv = nc.dram_tensor("Wv", [D, H], f32, kind="ExternalInput")
    Wo = nc.dram_tensor("Wo", [H, 2], f32, kind="ExternalInput")
    bo = nc.dram_tensor("bo", [1, 2], f32, kind="ExternalInput")
    Wg1 = nc.dram_tensor("Wg1", [D, D], f32, kind="ExternalInput")
    Wg2 = nc.dram_tensor("Wg2", [D, D], f32, kind="ExternalInput")
    Wb1 = nc.dram_tensor("Wb1", [D, D], f32, kind="ExternalInput")
    Wb2 = nc.dram_tensor("Wb2", [D, D], f32, kind="ExternalInput")
    y = nc.dram_tensor("y", [SSH, 2], f32, kind="ExternalOutput")

    # DRAM scratch for partition-broadcast bounces
    k_dram = nc.dram_tensor("k_dram", [H, S], bf16)
    v_dram = nc.dram_tensor("v_dram", [H, S], bf16)

    with tile.TileContext(nc) as tc:
        with tc.tile_pool(name="big", bufs=1) as big, \
             tc.tile_pool(name="scr", bufs=4) as scrp, \
             tc.tile_pool(name="kv", bufs=1) as kvp, \
             tc.tile_pool(name="small", bufs=1) as sm, \
             tc.tile_pool(name="att", bufs=3) as att, \
             tc.tile_pool(name="psum", bufs=1, space="PSUM") as psum:

            # ---------- loads ----------
            x2big = big.tile([P, TC * D], f32)
            nc.sync.dma_start(x2big[:], x2.rearrange("(p c) d -> p (c d)", p=P))
            x1fbig = big.tile([P, TC * D], f32)
            nc.sync.dma_start(x1fbig[:], x1f.rearrange("(p c) d -> p (c d)", p=P))
            x1sbig = big.tile([P, SC * D], f32)
            nc.sync.dma_start(x1sbig[:], x1s.rearrange("(p c) d -> p (c d)", p=P))

            wg1 = sm.tile([D, D], f32)
            nc.scalar.dma_start(wg1[:], Wg1[:, :])
            wg2 = sm.tile([D, D], f32)
            nc.scalar.dma_start(wg2[:], Wg2[:, :])
            wb1 = sm.tile([D, D], f32)
            nc.scalar.dma_start(wb1[:], Wb1[:, :])
            wb2 = sm.tile([D, D], f32)
            nc.scalar.dma_start(wb2[:], Wb2[:, :])

            # All small per-partition broadcasts (qkv weight columns, Wo, bo)
            # built on PE: transpose each [64,2] weight to rows, then a small
            # ones-matmul per row broadcasts it into a slice of one PSUM tile.
            # Avoids ~15 fixed-cost DMAs through DRAM.
            from concourse.masks import make_identity
            ident = sm.tile([P, P], f32)
            make_identity(nc, ident[:])
            ones_r = sm.tile([1, P], f32)
            nc.vector.memset(ones_r[:], 1.0)
            # sel[h]: [2,128] with row h all-ones -> lhsT.T @ twr picks row h
            sel0 = sm.tile([H, P], f32)
            nc.vector.memset(sel0[:], 0.0)
            nc.vector.memset(sel0[0:1, :], 1.0)
            sel1 = sm.tile([H, P], f32)
            nc.vector.memset(sel1[:], 1.0)
            nc.vector.memset(sel1[0:1, :], 0.0)
            sel = [sel0, sel1]
            wab_ps = psum.tile([P, 6 * D + 6], f32)
            for i, Wt in enumerate((Wq, Wk, Wv)):
                t = sm.tile([D, H], f32, name=f"wtmp{i}")
                nc.scalar.dma_start(t[:], Wt[:, :])
                tp = psum.tile([H, D], f32, name=f"wtp{i}", tag="wtp")
                nc.tensor.transpose(tp[:], t[:], ident[:D, :D])
                twr = sm.tile([H, D], f32, name=f"twr{i}")
                nc.vector.tensor_copy(twr[:], tp[:])
                for h in range(H):
                    nc.tensor.matmul(wab_ps[:, (2 * i + h) * D:(2 * i + h + 1) * D],
                                     sel[h][:], twr[:], start=True, stop=True)
            wof = sm.tile([1, 4], f32)
            nc.scalar.dma_start(wof[:], Wo.rearrange("h j -> (h j)").rearrange("(o f) -> o f", o=1))
            nc.tensor.matmul(wab_ps[:, 6 * D:6 * D + 4], ones_r[:], wof[:], start=True, stop=True)
            bof = sm.tile([1, 2], f32)
            nc.scalar.dma_start(bof[:], bo[:, :])
            nc.tensor.matmul(wab_ps[:, 6 * D + 4:6 * D + 6], ones_r[:], bof[:], start=True, stop=True)
            wab = sm.tile([P, 6 * D + 6], f32)
            nc.vector.tensor_copy(wab[:], wab_ps[:])
            wq_b = [wab[:, h * D:(h + 1) * D] for h in range(H)]
            wk_b = [wab[:, (2 + h) * D:(3 + h) * D] for h in range(H)]
            wv_b = [wab[:, (4 + h) * D:(5 + h) * D] for h in range(H)]
            wo_b = {(h, j): wab[:, 6 * D + h * 2 + j:6 * D + h * 2 + j + 1]
                    for h in range(H) for j in range(2)}
            bo_b = [wab[:, 6 * D + 4 + j:6 * D + 4 + j + 1] for j in range(2)]

            ones = sm.tile([P, 1], f32)
            nc.vector.memset(ones[:], 1.0)

            # ---------- x1 mean -> h_col [64,1] ----------
            h_ps = psum.tile([D, 1], f32)
            for c in range(TC):
                nc.tensor.matmul(h_ps[:], x1fbig[:, c * D:(c + 1) * D], ones[:],
                                 start=(c == 0), stop=(c == TC - 1))
            h_col = sm.tile([D, 1], f32)
            nc.vector.tensor_scalar_mul(h_col[:], h_ps[:], 1.0 / S)

            # ---------- x2 stats: mu, E[x^2] ----------
            x2sq = big.tile([P, TC * D], f32)
            nc.gpsimd.tensor_tensor(out=x2sq[:], in0=x2big[:], in1=x2big[:], op=ALU.mult)

            mu_ps = psum.tile([1, D], f32)
            for c in range(TC):
                nc.tensor.matmul(mu_ps[:], ones[:], x2big[:, c * D:(c + 1) * D],
                                 start=(c == 0), stop=(c == TC - 1))
            msq_ps = psum.tile([1, D], f32)
            for c in range(TC):
                nc.tensor.matmul(msq_ps[:], ones[:], x2sq[:, c * D:(c + 1) * D],
                                 start=(c == 0), stop=(c == TC - 1))
            mu = sm.tile([1, D], f32)
            nc.vector.tensor_scalar_mul(mu[:], mu_ps[:], 1.0 / S)
            msq = sm.tile([1, D], f32)
            nc.vector.tensor_scalar_mul(msq[:], msq_ps[:], 1.0 / S)

            # var = msq - mu^2 ; rstd = sqrt(1/(var+eps))
            musq = sm.tile([1, D], f32)
            nc.vector.tensor_tensor(out=musq[:], in0=mu[:], in1=mu[:], op=ALU.mult)
            var = sm.tile([1, D], f32)
            nc.vector.tensor_tensor(out=var[:], in0=msq[:], in1=musq[:], op=ALU.subtract)
            nc.vector.tensor_scalar_add(var[:], var[:], EPS_BN)
            rvar = sm.tile([1, D], f32)
            nc.vector.reciprocal(rvar[:], var[:])
            rstd = sm.tile([1, D], f32)
            nc.scalar.activation(rstd[:], rvar[:], AF.Sqrt)

            # ---------- CBN MLPs: dg, db rows [1, 64] ----------
            def mlp(w1, w2, name):
                z_ps = psum.tile([D, 1], f32, name=f"z_ps_{name}", tag="z_ps")
                nc.tensor.matmul(z_ps[:], w1[:], h_col[:], start=True, stop=True)
                zr = sm.tile([D, 1], f32, name=f"zr_{name}")
                nc.scalar.activation(zr[:], z_ps[:], AF.Relu)
                d_ps = psum.tile([1, D], f32, name=f"d_ps_{name}", tag="d_ps")
                nc.tensor.matmul(d_ps[:], zr[:], w2[:], start=True, stop=True)
                return d_ps

            dg_ps = mlp(wg1, wg2, "g")
            db_ps = mlp(wb1, wb2, "b")

            # A = (1+dg)*rstd ; B = db - mu*A
            dgp1 = sm.tile([1, D], f32)
            nc.vector.tensor_scalar_add(dgp1[:], dg_ps[:], 1.0)
            A_row = sm.tile([1, D], f32)
            nc.vector.tensor_tensor(out=A_row[:], in0=dgp1[:], in1=rstd[:], op=ALU.mult)
            muA = sm.tile([1, D], f32)
            nc.vector.tensor_tensor(out=muA[:], in0=mu[:], in1=A_row[:], op=ALU.mult)
            B_row = sm.tile([1, D], f32)
            nc.vector.tensor_tensor(out=B_row[:], in0=db_ps[:], in1=muA[:], op=ALU.subtract)

            # bounce A,B through DRAM to partition-broadcast
            # broadcast A,B across partitions on PE: out = ones[1,128].T @ ab_row[1,128]
            ab_row = sm.tile([1, 2 * D], f32)
            nc.vector.tensor_copy(ab_row[:, 0:D], A_row[:])
            nc.vector.tensor_copy(ab_row[:, D:2 * D], B_row[:])
            ab_ps = psum.tile([P, 2 * D], f32)
            nc.tensor.matmul(ab_ps[:], ones_r[:], ab_row[:], start=True, stop=True)
            ab_b = sm.tile([P, 2 * D], f32)
            nc.vector.tensor_copy(ab_b[:], ab_ps[:])
            A_b = ab_b[:, 0:D]
            B_b = ab_b[:, D:2 * D]

            # ---------- v2 = x2*A + B (CBN applied) ----------
            v2 = big.tile([P, TC * D], f32)
            x2v = x2big[:].rearrange("p (c d) -> p c d", d=D)
            v2v = v2[:].rearrange("p (c d) -> p c d", d=D)
            A_rep = A_b.rearrange("p (c d) -> p c d", c=1).to_broadcast((P, TC, D))
            B_rep = B_b.rearrange("p (c d) -> p c d", c=1).to_broadcast((P, TC, D))
            nc.gpsimd.tensor_tensor(out=v2v, in0=x2v, in1=A_rep, op=ALU.mult)
            nc.gpsimd.tensor_tensor(out=v2v, in0=v2v, in1=B_rep, op=ALU.add)

            # ---------- row norms ----------
            rn2 = sm.tile([P, TC], f32)
            nc.vector.reduce_sum(rn2[:], x2sq[:].rearrange("p (c d) -> p c d", d=D),
                                 axis=mybir.AxisListType.X)
            in2 = sm.tile([P, TC], f32)
            nc.vector.reciprocal(in2[:], rn2[:])
            nc.scalar.activation(in2[:], in2[:], AF.Sqrt)

            v2sq = big.tile([P, TC * D], f32)
            nc.gpsimd.tensor_tensor(out=v2sq[:], in0=v2[:], in1=v2[:], op=ALU.mult)
            rnv = sm.tile([P, TC], f32)
            nc.vector.reduce_sum(rnv[:], v2sq[:].rearrange("p (c d) -> p c d", d=D),
                                 axis=mybir.AxisListType.X)
            inv2 = sm.tile([P, TC], f32)
            nc.vector.reciprocal(inv2[:], rnv[:])
            nc.scalar.activation(inv2[:], inv2[:], AF.Sqrt)

            x1ssq = sm.tile([P, SC * D], f32)
            nc.vector.tensor_tensor(out=x1ssq[:], in0=x1sbig[:], in1=x1sbig[:], op=ALU.mult)
            rn1 = sm.tile([P, SC], f32)
            nc.vector.reduce_sum(rn1[:], x1ssq[:].rearrange("p (c d) -> p c d", d=D),
                                 axis=mybir.AxisListType.X)
            in1 = sm.tile([P, SC], f32)
            nc.vector.reciprocal(in1[:], rn1[:])
            nc.scalar.activation(in1[:], in1[:], AF.Sqrt)

            # ---------- projections (col-form mul+reduce) ----------
            def proj(src_big, w_b, inv, n_chunks, name, out_dt=f32, meng=None):
                meng = meng or nc.vector
                """out[p, c] = inv[p,c] * sum_d src[p, c, d] * w_b[p, d]"""
                scr = scrp.tile([P, n_chunks * D], f32, tag="scr", name=f"scr_{name}")
                w_rep = w_b.rearrange("p (c d) -> p c d", c=1).to_broadcast((P, n_chunks, D))
                meng.tensor_tensor(out=scr[:].rearrange("p (c d) -> p c d", d=D),
                                   in0=src_big[:].rearrange("p (c d) -> p c d", d=D),
                                   in1=w_rep, op=ALU.mult)
                raw = sm.tile([P, n_chunks], f32, name=f"raw_{name}")
                nc.vector.reduce_sum(raw[:], scr[:].rearrange("p (c d) -> p c d", d=D),
                                     axis=mybir.AxisListType.X)
                outp = sm.tile([P, n_chunks], out_dt, name=f"proj_{name}")
                nc.vector.tensor_tensor(out=outp[:], in0=raw[:], in1=inv[:], op=ALU.mult)
                return outp

            q_hat = [proj(x1sbig, wq_b[h], in1, SC, f"q{h}") for h in range(H)]
            k_hat = [proj(x2big, wk_b[h], in2, TC, f"k{h}", out_dt=bf16, meng=nc.gpsimd) for h in range(H)]
            v_hat = [proj(v2, wv_b[h], inv2, TC, f"v{h}", out_dt=bf16) for h in range(H)]

            # ---------- broadcast k and v (bf16) across partitions ----------
            # bf16 halves the broadcast bytes; spread across SP/PE HW queues.
            k_b = []
            v_b = []
            for h in range(H):
                nc.sync.dma_start(k_dram[h:h + 1, :], k_hat[h][:])
                nc.sync.dma_start(v_dram[h:h + 1, :], v_hat[h][:])
                kb = kvp.tile([P, S], bf16, name=f"k_b{h}")
                eng = nc.sync if h == 0 else nc.gpsimd
                eng.dma_start(kb[:], k_dram[h:h + 1, :].to_broadcast((P, S)))
                k_b.append(kb)
                vb = kvp.tile([P, S], bf16, name=f"v_b{h}")
                eng = nc.sync if h == 0 else nc.gpsimd
                eng.dma_start(vb[:], v_dram[h:h + 1, :].to_broadcast((P, S)))
                v_b.append(vb)

            # ---------- attention: per (head, s-chunk) ----------
            den_all = sm.tile([P, H * SC], f32)
            num_all = sm.tile([P, H * SC], f32)
            for h in range(H):
                for sc in range(SC):
                    idx = h * SC + sc
                    e_t = att.tile([P, S], bf16, tag="e", name=f"e_{h}_{sc}")
                    nc.scalar.activation(e_t[:], k_b[h][:], AF.Exp,
                                         bias=0.0, scale=q_hat[h][:, sc:sc + 1],
                                         accum_out=den_all[:, idx:idx + 1])
                    scr = att.tile([P, S], bf16, tag="ttr_scr", name=f"ts_{h}_{sc}")
                    nc.vector.tensor_tensor(out=scr[:], in0=e_t[:], in1=v_b[h][:], op=ALU.mult)
                    h1 = att.tile([P, S // 2], bf16, tag="h1", name=f"h1_{h}_{sc}")
                    nc.vector.tensor_tensor(out=h1[:], in0=scr[:, :S // 2], in1=scr[:, S // 2:], op=ALU.add)
                    h2 = att.tile([P, S // 4], bf16, tag="h2", name=f"h2_{h}_{sc}")
                    nc.vector.tensor_tensor(out=h2[:], in0=h1[:, :S // 4], in1=h1[:, S // 4:], op=ALU.add)
                    nc.vector.reduce_sum(num_all[:, idx:idx + 1], h2[:], axis=mybir.AxisListType.X)

            # ---------- epilogue: batched r, logits, sigmoid ----------
            rden_all = sm.tile([P, H * SC], f32)
            nc.vector.reciprocal(rden_all[:], den_all[:])
            r_all = sm.tile([P, H * SC], f32)
            nc.vector.tensor_tensor(out=r_all[:], in0=num_all[:], in1=rden_all[:], op=ALU.mult)
            r0 = r_all[:, 0:SC]
            r1 = r_all[:, SC:2 * SC]

            z_all = sm.tile([P, SC * 2], f32)
            zv = z_all[:].rearrange("p (c j) -> p c j", j=2)
            t2 = sm.tile([P, SC * 2], f32)
            t2v = t2[:].rearrange("p (c j) -> p c j", j=2)
            for j in range(2):
                nc.vector.tensor_scalar(out=zv[:, :, j], in0=r0,
                                        scalar1=wo_b[(0, j)], scalar2=bo_b[j],
                                        op0=ALU.mult, op1=ALU.add)
                nc.vector.tensor_scalar_mul(t2v[:, :, j], r1, wo_b[(1, j)])
            nc.vector.tensor_tensor(out=z_all[:], in0=z_all[:], in1=t2[:], op=ALU.add)

            # sigmoid(z) = 1/(1+exp(-z))  (reuses the exp table set)
            sig = sm.tile([P, SC * 2], f32)
            nc.scalar.activation(sig[:], z_all[:], AF.Exp, bias=0.0, scale=-1.0)
            nc.vector.tensor_scalar_add(sig[:], sig[:], 1.0)
            nc.vector.reciprocal(sig[:], sig[:])

            nc.sync.dma_start(y.rearrange("(p c) j -> p (c j)", p=P), sig[:])

    if split:
        _split_waits(nc, mybir)
    return nc


def _split_waits(nc, mybir, maxw=1):
    """This container's walrus build rejects instructions carrying more than
    ~2 sync-wait commands. Split excess waits onto zero-register-write nops
    inserted just before the instruction on the same engine (same-engine
    program order preserves the wait-before-execute semantics)."""
    ctr = 0
    for bb in nc.m.functions[0].blocks:
        new = []
        for inst in bb.instructions:
            si = inst.sync_info
            if si is not None and si.on_wait and len(si.on_wait) > maxw:
                waits = list(si.on_wait)
                ename = str(inst.engine).split(".")[-1]
                for w in waits[:-maxw]:
                    ctr += 1
                    new.append(mybir.InstRegisterMove(
                        name=f"WS-{ctr}",
                        ins=[mybir.ImmediateValue(kind="imm_value", dtype=mybir.dt.int32, value=0)],
                        outs=[mybir.RegisterAccess(kind="register_access", regref=f"{ename}_zero", dtype=mybir.dt.int32)],
                        engine=inst.engine,
                        sync_info=mybir.SyncInfo(on_wait=[w], on_update=[]),
                    ))
                si.on_wait = waits[-maxw:]
            new.append(inst)
        bb.instructions = new


def _get_program():
    if "nc" not in _CACHE:
        _CACHE["nc"] = _build()
    return _CACHE["nc"]


def kernel(x1, x2, Wq, Wk, Wv, Wo, bo, Wg1, Wg2, Wb1, Wb2):
    from concourse import bass_utils

    nc = _get_program()
    x1s_full = np.ascontiguousarray(x1[0])  # [4096, 64]
    x2s = np.ascontiguousarray(x2[0])

    in_maps = []
    for i in range(NCORES):
        in_maps.append({
            "x1s": np.ascontiguousarray(x1s_full[i * SSH:(i + 1) * SSH]),
            "x1f": x1s_full,
            "x2": x2s,
            "Wq": Wq, "Wk": Wk, "Wv": Wv, "Wo": Wo,
            "bo": np.ascontiguousarray(bo[None, :]),
            "Wg1": Wg1, "Wg2": Wg2, "Wb1": Wb1, "Wb2": Wb2,
        })

    # First execution of a freshly-compiled NEFF occasionally reports a
    # transient device error through the PJRT proxy; a retry succeeds.
    last_err = None
    for attempt in range(3):
        try:
            res = bass_utils.run_bass_kernel_spmd(nc, in_maps, core_ids=list(range(NCORES)))
            out = np.concatenate([res.results[i]["y"] for i in range(NCORES)], axis=0)
            return out.reshape(1, S, 2)
        except Exception as e:  # noqa: BLE001
            last_err = e
            import time
            time.sleep(5)
    raise last_err
